# revision 3
# baseline (speedup 1.0000x reference)
"""Local (banded, window=3) attention TRN2 kernel.

Full-input contract: kernel(**inputs) takes the complete tensors
  x [8, 1024, 384], qkv_w [1152, 384], proj_w [384, 384], proj_b [384]
and returns the full output [8, 1024, 384].

Sharding: data-parallel over batch B=8 -> one batch element per NeuronCore.

Per-core algorithm (everything on chip in fp32, matmuls in fp32r):
  xT [C=384, N=1024] (host-pretransposed shard)
  qkvT[ch, t] = qkv_w @ x_b.T           (PE; lhsT = host-pretransposed qkv_w.T)
  band scores s_off[h, t] = sum_d q[(h,d),t] * k[(h,d),t+off], off in {-1,0,+1}
     products on DVE in [ch, t] layout (token shift = free-dim slice),
     partition-reduction over d via PE matmul against a 0/1 head-indicator
  p = softmax over the 3 offsets (ACT exp, DVE add/recip/mul on [6, 1024])
  attn_outT[(h,d), t] = sum_off pbcast_off[(h,d), t] * vT[(h,d), t+off]
     (p broadcast head->64 rows via PE indicator matmul, multiply-add on DVE)
  yT = proj_w @ attn_outT + b           (PE; bias folded in as a K=1 matmul)
Host transposes yT back to [1024, 384] per batch element.
"""

import numpy as np

B, N, C = 8, 1024, 384
H, HD = 6, 64
CQKV = 3 * C  # 1152
NCORES = 8
P = 128
NHALF = N // 2  # 512
KC = C // P  # 3 contraction chunks

_cached = {}


def _build_nc():
    import concourse.bacc as bacc
    import concourse.tile as tile
    from concourse import mybir

    f32 = mybir.dt.float32
    f32r = mybir.dt.float32r
    AF = mybir.ActivationFunctionType

    nc = bacc.Bacc("TRN2", target_bir_lowering=False, debug=False,
                   num_devices=NCORES)

    d_xT = nc.dram_tensor("xT", [C, N], f32r, kind="ExternalInput").ap()
    d_wqkvT = nc.dram_tensor("qkv_wT", [C, CQKV], f32r, kind="ExternalInput").ap()
    d_wprojT = nc.dram_tensor("proj_wT", [C, C], f32r, kind="ExternalInput").ap()
    d_bias = nc.dram_tensor("proj_b", [1, C], f32r, kind="ExternalInput").ap()
    d_ind6 = nc.dram_tensor("ind6", [P, 6 * KC], f32r, kind="ExternalInput").ap()
    d_ind6T = nc.dram_tensor("ind6T", [H, P * KC], f32r, kind="ExternalInput").ap()
    d_ones = nc.dram_tensor("ones", [1, N], f32r, kind="ExternalInput").ap()
    d_yT = nc.dram_tensor("yT", [C, N], f32, kind="ExternalOutput").ap()

    def r(ap):
        return ap.bitcast(f32r)

    with tile.TileContext(nc) as tc:
        import contextlib
        with contextlib.ExitStack() as ctx:
            wpool = ctx.enter_context(tc.tile_pool(name="w", bufs=1))
            xpool = ctx.enter_context(tc.tile_pool(name="x", bufs=1))
            qkvpool = ctx.enter_context(tc.tile_pool(name="qkv", bufs=1))
            prodpool = ctx.enter_context(tc.tile_pool(name="prod", bufs=4))
            avpool = ctx.enter_context(tc.tile_pool(name="av", bufs=4))
            aopool = ctx.enter_context(tc.tile_pool(name="ao", bufs=1))
            ypool = ctx.enter_context(tc.tile_pool(name="y", bufs=3))
            spool = ctx.enter_context(
                tc.tile_pool(name="s", bufs=4, space="PSUM"))
            mmpool = ctx.enter_context(
                tc.tile_pool(name="mm", bufs=4, space="PSUM"))
            epool = ctx.enter_context(tc.tile_pool(name="e", bufs=10))

            # ---- constant / weight loads ----
            w_qkv = []
            w_proj = []
            x_t = []
            for kc in range(KC):
                wt = wpool.tile([P, CQKV], f32r, name=f"wqkv{kc}")
                nc.sync.dma_start(out=wt, in_=d_wqkvT[P * kc:P * (kc + 1), :])
                w_qkv.append(wt)
                pt = wpool.tile([P, C], f32r, name=f"wproj{kc}")
                nc.sync.dma_start(out=pt, in_=d_wprojT[P * kc:P * (kc + 1), :])
                w_proj.append(pt)
                xt = xpool.tile([P, N], f32r, name=f"xT{kc}")
                nc.sync.dma_start(out=xt, in_=d_xT[P * kc:P * (kc + 1), :])
                x_t.append(xt)
            ind6 = wpool.tile([P, 6 * KC], f32r, name="ind6")
            nc.sync.dma_start(out=ind6, in_=d_ind6)
            ind6T = wpool.tile([H, P * KC], f32r, name="ind6T")
            nc.sync.dma_start(out=ind6T, in_=d_ind6T)
            bias = wpool.tile([1, C], f32r, name="bias")
            nc.sync.dma_start(out=bias, in_=d_bias)
            ones = wpool.tile([1, N], f32r, name="ones")
            nc.sync.dma_start(out=ones, in_=d_ones)

            # ---- stage 1: qkvT[m] [128, 1024] for m in 0..8 ----
            qkvT = []
            for m in range(CQKV // P):  # 9 chunks: 0-2 q, 3-5 k, 6-8 v
                qt = qkvpool.tile([P, N], f32r, name=f"qkvT{m}")
                for h in range(2):
                    ps = mmpool.tile([P, NHALF], f32, tag="mm")
                    for kc in range(KC):
                        nc.tensor.matmul(
                            ps,
                            lhsT=(w_qkv[kc][:, P * m:P * (m + 1)]),
                            rhs=(x_t[kc][:, NHALF * h:NHALF * (h + 1)]),
                            start=(kc == 0), stop=(kc == KC - 1),
                        )
                    dst = qt[:, NHALF * h:NHALF * (h + 1)]
                    if m < 6:
                        nc.scalar.copy(dst, ps)
                    else:
                        nc.vector.tensor_copy(dst, ps)
                qkvT.append(qt)

            # ---- banded score products (DVE) + partition reduce (PE) ----
            # offsets: 0 -> j=t-1 (left), 1 -> j=t (center), 2 -> j=t+1 (right)
            prods = [[None] * KC for _ in range(3)]
            for kc in range(KC):
                q = qkvT[kc]
                k = qkvT[3 + kc]
                pl = prodpool.tile([P, N], f32r, tag="prod")
                # col 0 left unwritten: masked after exp via e_off[0][:, 0]
                nc.vector.tensor_mul(pl[:, 1:N], q[:, 1:N], k[:, 0:N - 1])
                prods[0][kc] = pl
                pc = prodpool.tile([P, N], f32r, tag="prod")
                nc.vector.tensor_mul(pc, q, k)
                prods[1][kc] = pc
                pr = prodpool.tile([P, N], f32r, tag="prod")
                # col N-1 left unwritten: masked after exp via e_off[2][:, N-1]
                nc.vector.tensor_mul(pr[:, 0:N - 1], q[:, 0:N - 1], k[:, 1:N])
                prods[2][kc] = pr

            e_off = []
            for off in range(3):
                et = epool.tile([H, N], f32, tag="e", name=f"e{off}")
                for h in range(2):
                    sps = spool.tile([H, NHALF], f32, tag="s")
                    for kc in range(KC):
                        nc.tensor.matmul(
                            sps,
                            lhsT=(ind6[:, 6 * kc:6 * (kc + 1)]),
                            rhs=(prods[off][kc][:, NHALF * h:NHALF * (h + 1)]),
                            start=(kc == 0), stop=(kc == KC - 1),
                        )
                    nc.scalar.activation(
                        et[:, NHALF * h:NHALF * (h + 1)], sps, AF.Exp,
                        scale=float(HD) ** -0.5)
                e_off.append(et)

            # boundary masking: no left neighbor at t=0, no right at t=N-1
            nc.gpsimd.memset(e_off[0][:, 0:1], 0.0)
            nc.gpsimd.memset(e_off[2][:, N - 1:N], 0.0)

            # ---- softmax over the 3 offsets ----
            den0 = epool.tile([H, N], f32, tag="e")
            nc.vector.tensor_add(den0, e_off[0], e_off[1])
            den = epool.tile([H, N], f32, tag="e")
            nc.vector.tensor_add(den, den0, e_off[2])
            rec = epool.tile([H, N], f32, tag="e")
            nc.vector.reciprocal(rec, den)
            p_off = []
            for off in range(3):
                pt = epool.tile([H, N], f32r, tag="e", name=f"p{off}")
                nc.vector.tensor_mul(pt, e_off[off], rec)
                p_off.append(pt)

            # ---- broadcast p to 64 rows/head (PE) + AV multiply-add (DVE) ----
            attn = []
            for kc in range(KC):
                v = qkvT[6 + kc]
                pb_sb = []
                for off in range(3):
                    pb = avpool.tile([P, N], f32, tag="pb")
                    for h in range(2):
                        pbps = mmpool.tile([P, NHALF], f32, tag="mm")
                        nc.tensor.matmul(
                            pbps,
                            lhsT=(ind6T[:, P * kc:P * (kc + 1)]),
                            rhs=(p_off[off][:, NHALF * h:NHALF * (h + 1)]),
                            start=True, stop=True,
                        )
                        nc.scalar.copy(pb[:, NHALF * h:NHALF * (h + 1)], pbps)
                    pb_sb.append(pb)
                m_c = avpool.tile([P, N], f32, tag="m")
                nc.vector.tensor_mul(m_c, pb_sb[1], v)
                m_l = avpool.tile([P, N], f32, tag="m")
                nc.gpsimd.memset(m_l[:, 0:1], 0.0)
                nc.vector.tensor_mul(m_l[:, 1:N], pb_sb[0][:, 1:N],
                                     v[:, 0:N - 1])
                m_r = avpool.tile([P, N], f32, tag="m")
                nc.gpsimd.memset(m_r[:, N - 1:N], 0.0)
                nc.vector.tensor_mul(m_r[:, 0:N - 1], pb_sb[2][:, 0:N - 1],
                                     v[:, 1:N])
                s01 = avpool.tile([P, N], f32, tag="m")
                nc.vector.tensor_add(s01, m_c, m_l)
                ao = aopool.tile([P, N], f32r, name=f"attn{kc}")
                nc.vector.tensor_add(ao, s01, m_r)
                attn.append(ao)

            # ---- output projection + bias ----
            for m in range(KC):
                for h in range(2):
                    yps = mmpool.tile([P, NHALF], f32, tag="mm")
                    for kc in range(KC):
                        nc.tensor.matmul(
                            yps,
                            lhsT=(w_proj[kc][:, P * m:P * (m + 1)]),
                            rhs=(attn[kc][:, NHALF * h:NHALF * (h + 1)]),
                            start=(kc == 0), stop=False,
                        )
                    nc.tensor.matmul(
                        yps,
                        lhsT=(bias[:, P * m:P * (m + 1)]),
                        rhs=(ones[:, NHALF * h:NHALF * (h + 1)]),
                        start=False, stop=True,
                    )
                    yt = ypool.tile([P, NHALF], f32, tag="y")
                    nc.scalar.copy(yt, yps)
                    nc.sync.dma_start(
                        out=d_yT[P * m:P * (m + 1),
                                 NHALF * h:NHALF * (h + 1)],
                        in_=yt)

    nc.compile()
    return nc


def _host_inputs(x, qkv_w, proj_w, proj_b):
    qkv_wT = np.ascontiguousarray(qkv_w.T.astype(np.float32))
    proj_wT = np.ascontiguousarray(proj_w.T.astype(np.float32))
    bias = np.ascontiguousarray(proj_b.astype(np.float32).reshape(1, C))
    # head indicator: row p of chunk kc belongs to head 2*kc + p//64
    ind6 = np.zeros((P, 6 * KC), np.float32)
    for kc in range(KC):
        for p in range(P):
            ind6[p, 6 * kc + 2 * kc + p // HD] = 1.0
    ind6T = np.zeros((H, P * KC), np.float32)
    for kc in range(KC):
        for p in range(P):
            ind6T[2 * kc + p // HD, P * kc + p] = 1.0
    shared = {
        "ones": np.ones((1, N), np.float32),
        "qkv_wT": qkv_wT,
        "proj_wT": proj_wT,
        "proj_b": bias,
        "ind6": ind6,
        "ind6T": ind6T,
    }
    in_maps = []
    for b in range(B):
        m = dict(shared)
        m["xT"] = np.ascontiguousarray(x[b].astype(np.float32).T)
        in_maps.append(m)
    return in_maps


def kernel(x, qkv_w, proj_w, proj_b, _trace=False):
    from concourse import bass_utils

    x = np.asarray(x)
    if "nc" not in _cached:
        _cached["nc"] = _build_nc()
    nc = _cached["nc"]
    in_maps = _host_inputs(x, np.asarray(qkv_w), np.asarray(proj_w),
                           np.asarray(proj_b))
    res = bass_utils.run_bass_kernel_spmd(
        nc, in_maps, core_ids=list(range(NCORES)), trace=_trace)
    out = np.empty((B, N, C), np.float32)
    for b in range(B):
        out[b] = res.results[b]["yT"].T
    if _trace:
        _cached["last_result"] = res
    return out


# revision 9
# speedup vs baseline: 1.5225x; 1.5225x over previous
"""Local (banded, window=3) attention TRN2 kernel.

Full-input contract: kernel(**inputs) takes the complete tensors
  x [8, 1024, 384], qkv_w [1152, 384], proj_w [384, 384], proj_b [384]
and returns the full output [8, 1024, 384].

Sharding: data-parallel over batch B=8 -> one batch element per NeuronCore.

Per-core algorithm (bf16 data, fp32 PSUM accumulation, fp32 softmax):
  xT [C=384, N=1024] (host-pretransposed shard, bf16)
  qkvT[ch, t] = qkv_w @ x_b.T          (PE; lhsT = host-pretransposed qkv_w.T)
  band scores s_off[h, t] = sum_d q[(h,d),t] * k[(h,d),t+off], off in {-1,0,+1}
     products on DVE in [ch, t] layout (token shift = free-dim slice),
     partition-reduction over d via PE matmul against a 0/1 head-indicator
  p = softmax over the 3 offsets (ACT exp, DVE add / recip-approx / mul)
  attn_outT[(h,d), t] = sum_off pbcast_off[(h,d), t] * vT[(h,d), t+off]
     (p broadcast head->64 rows via PE indicator matmul into PSUM,
      multiply-add on DVE reading PSUM directly)
  yT = proj_w @ attn_outT + b     (PE; bias folded in as a K=1 matmul)
Host transposes yT back to [1024, 384] fp32 per batch element.
"""

import numpy as np

B, N, C = 8, 1024, 384
H, HD = 6, 64
CQKV = 3 * C  # 1152
NCORES = 8
P = 128
NHALF = N // 2  # 512
KC = C // P  # 3 contraction chunks

_cached = {}


def _build_nc():
    import contextlib

    import concourse.bacc as bacc
    import concourse.tile as tile
    from concourse import mybir

    f32 = mybir.dt.float32
    bf16 = mybir.dt.bfloat16
    AF = mybir.ActivationFunctionType

    nc = bacc.Bacc("TRN2", target_bir_lowering=False, debug=False,
                   num_devices=NCORES)

    d_xT = nc.dram_tensor("xT", [C, N], bf16, kind="ExternalInput").ap()
    d_wqkvT = nc.dram_tensor("qkv_wT", [C, CQKV], bf16,
                             kind="ExternalInput").ap()
    d_wprojT = nc.dram_tensor("proj_wT", [C, C], bf16,
                              kind="ExternalInput").ap()
    d_bias = nc.dram_tensor("proj_b", [1, C], bf16, kind="ExternalInput").ap()
    d_ind6 = nc.dram_tensor("ind6", [P, 6 * KC], bf16,
                            kind="ExternalInput").ap()
    d_ind6T = nc.dram_tensor("ind6T", [H, P * KC], bf16,
                             kind="ExternalInput").ap()
    d_ones = nc.dram_tensor("ones", [1, N], bf16, kind="ExternalInput").ap()
    d_yT = nc.dram_tensor("yT", [C, N], bf16, kind="ExternalOutput").ap()

    with tile.TileContext(nc) as tc, contextlib.ExitStack() as ctx:
        wpool = ctx.enter_context(tc.tile_pool(name="w", bufs=1))
        xpool = ctx.enter_context(tc.tile_pool(name="x", bufs=1))
        qkvpool = ctx.enter_context(tc.tile_pool(name="qkv", bufs=1))
        prodpool = ctx.enter_context(tc.tile_pool(name="prod", bufs=12))
        avpool = ctx.enter_context(tc.tile_pool(name="av", bufs=10))
        aopool = ctx.enter_context(tc.tile_pool(name="ao", bufs=1))
        ypool = ctx.enter_context(tc.tile_pool(name="y", bufs=4))
        epool = ctx.enter_context(tc.tile_pool(name="e", bufs=20))
        # PSUM budget (8 banks of 512 fp32):
        #   mm   [128, 512] = 1 bank x 4 bufs = 4  (stage-1 qkv)
        #   pb   [128, 512]  = 1 bank  x 2 bufs = 2  (p-broadcast + proj)
        #   s    [6, 512]    = 1 bank  x 2 bufs = 2  (scores)
        mmpool = ctx.enter_context(
            tc.tile_pool(name="mm", bufs=4, space="PSUM"))
        pbpool = ctx.enter_context(
            tc.tile_pool(name="pb", bufs=2, space="PSUM"))
        spool = ctx.enter_context(
            tc.tile_pool(name="s", bufs=2, space="PSUM"))

        # ---- inputs: per-chunk DMAs on both HWDGE queues (sync + scalar)
        # so stage-1 matmuls of chunk kc can start as soon as x[kc]/w[kc] land
        w_qkv, w_proj, x_t = [], [], []
        for kc in range(KC):
            xt = xpool.tile([P, N], bf16, name=f"xT{kc}")
            if kc == 0:
                # split so the first stage-1 matmul group starts sooner
                nc.sync.dma_start(out=xt[:, 0:NHALF],
                                  in_=d_xT[0:P, 0:NHALF])
                nc.sync.dma_start(out=xt[:, NHALF:N],
                                  in_=d_xT[0:P, NHALF:N])
            else:
                nc.sync.dma_start(out=xt, in_=d_xT[P * kc:P * (kc + 1), :])
            x_t.append(xt)
            wt = wpool.tile([P, CQKV], bf16, name=f"wqkv{kc}")
            # split per q/k/v part so the first stage-1 matmuls (q chunks)
            # start as soon as the q-part of the weights lands
            for part in range(3):
                nc.scalar.dma_start(
                    out=wt[:, C * part:C * (part + 1)],
                    in_=d_wqkvT[P * kc:P * (kc + 1), C * part:C * (part + 1)])
            w_qkv.append(wt)
        ind6 = wpool.tile([P, 6 * KC], bf16, name="ind6")
        nc.sync.dma_start(out=ind6, in_=d_ind6)
        ind6T = wpool.tile([H, P * KC], bf16, name="ind6T")
        nc.sync.dma_start(out=ind6T, in_=d_ind6T)
        for kc in range(KC):
            pt = wpool.tile([P, C], bf16, name=f"wproj{kc}")
            nc.scalar.dma_start(out=pt, in_=d_wprojT[P * kc:P * (kc + 1), :])
            w_proj.append(pt)
        bias = wpool.tile([1, C], bf16, name="bias")
        nc.sync.dma_start(out=bias, in_=d_bias)
        ones = wpool.tile([1, N], bf16, name="ones")
        nc.sync.dma_start(out=ones, in_=d_ones)

        def stage1_chunk(m, evac_engine):
            """qkvT[m] [128,1024] bf16 = (qkv_w @ x.T) rows 128m..128m+127."""
            qt = qkvpool.tile([P, N], bf16, name=f"qkvT{m}")
            for h in range(2):
                ps = mmpool.tile([P, NHALF], f32, tag="mm")
                for kc in range(KC):
                    nc.tensor.matmul(
                        ps,
                        lhsT=w_qkv[kc][:, P * m:P * (m + 1)],
                        rhs=x_t[kc][:, NHALF * h:NHALF * (h + 1)],
                        start=(kc == 0), stop=(kc == KC - 1),
                    )
                dst = qt[:, NHALF * h:NHALF * (h + 1)]
                nc.scalar.copy(dst, ps)
            return qt

        # ---- stage 1: all 9 qkvT chunks (PE warms up on a dense stream) ----
        # q/k interleaved so prods of chunk kc can start after 2 chunks;
        # v chunks are emitted later (after the score matmuls) so the PE has
        # work queued while the softmax chain runs on ACT/DVE
        qkvT = [None] * 9
        for m in (0, 3, 1, 4, 2, 5):
            qkvT[m] = stage1_chunk(m, "act")

        def half(ap, h):
            return ap[:, NHALF * h:NHALF * (h + 1)]


        # ---- banded attention mid-section, pipelined as 2 half-lanes ----
        # offsets: 0 -> key j=t-1, 1 -> j=t, 2 -> j=t+1
        def make_prod(off, kc):
            """prod[off][kc] [128, 1024] = q * shifted k (DVE, bf16)."""
            q = qkvT[kc]
            k = qkvT[3 + kc]
            pr = prodpool.tile([P, N], bf16, tag="prod",
                               name=f"prod{off}_{kc}")
            if off == 0:
                # col 0 unwritten: masked after exp via e[0] col 0
                nc.vector.tensor_mul(pr[:, 1:], q[:, 1:], k[:, 0:N - 1])
            elif off == 1:
                nc.vector.tensor_mul(pr, q, k)
            else:
                # col N-1 unwritten: masked after exp via e[2] col N-1
                nc.vector.tensor_mul(pr[:, 0:N - 1], q[:, 0:N - 1], k[:, 1:N])
            return pr

        prods = [[make_prod(off, kc) for kc in range(KC)] for off in range(3)]
        e_half = [[None] * 3 for _ in range(2)]   # [h][off]
        for h in range(2):
            for off in range(3):
                sps = spool.tile([H, NHALF], f32, tag="s")
                for kc in range(KC):
                    nc.tensor.matmul(
                        sps,
                        lhsT=ind6[:, 6 * kc:6 * (kc + 1)],
                        rhs=prods[off][kc][:, NHALF * h:NHALF * (h + 1)],
                        start=(kc == 0), stop=(kc == KC - 1),
                    )
                et = epool.tile([H, NHALF], f32, tag="e", name=f"e{h}_{off}")
                with tc.high_priority():
                    nc.scalar.activation(et, sps, AF.Exp,
                                         scale=float(HD) ** -0.5)
                e_half[h][off] = et

        # boundary masking: no left neighbor at t=0, no right at t=N-1
        nc.gpsimd.memset(e_half[0][0][:, 0:1], 0.0)
        nc.gpsimd.memset(e_half[1][2][:, NHALF - 1:NHALF], 0.0)

        # ---- stage 1 v chunks: PE work overlapping the softmax chain ----
        for m in (6, 7, 8):
            qkvT[m] = stage1_chunk(m, "act")

        # dL[t] = v[t-1] - v[t]  (padded: dL[0] = dL[N] = 0). AV then becomes
        # attn = v + p_l*dL - p_r*shift(dL), using that p_l + p_c + p_r = 1.
        dLs = []
        for kc in range(KC):
            v = qkvT[6 + kc]
            dL = avpool.tile([P, N + 1], bf16, tag="dv", name=f"dL{kc}")
            nc.vector.memset(dL[:, 0:1], 0.0)
            nc.vector.memset(dL[:, N:N + 1], 0.0)
            nc.vector.tensor_sub(dL[:, 1:N], v[:, 0:N - 1], v[:, 1:N])
            dLs.append(dL)

        # ---- softmax over the 3 offsets (per half) ----
        p_half = [[None] * 3 for _ in range(2)]
        for h in range(2):
            e0, e1, e2 = e_half[h]
            with tc.high_priority():
                den0 = epool.tile([H, NHALF], f32, tag="e")
                nc.vector.tensor_add(den0, e0, e1)
                den = epool.tile([H, NHALF], f32, tag="e")
                nc.vector.tensor_add(den, den0, e2)
                rec = epool.tile([H, NHALF], f32, tag="e")
                nc.vector.reciprocal_approx_fast(out=rec, in_=den)
                for off in (0, 2):
                    pt = epool.tile([H, NHALF], bf16, tag="p",
                                    name=f"p{h}_{off}")
                    nc.vector.tensor_mul(pt, e_half[h][off], rec)
                    p_half[h][off] = pt

        # ---- p broadcast (PE) + AV (DVE) + interleaved projection ----
        # proj accumulates over kc as soon as attn[kc] of this half exists,
        # so the PE overlaps the AV chain. yps tiles reuse the (now idle)
        # stage-1 mm PSUM pool.
        attn = [aopool.tile([P, N], bf16, name=f"attn{kc}")
                for kc in range(KC)]
        for h in range(2):
            lo = NHALF * h
            hi = lo + NHALF
            yps = [mmpool.tile([P, NHALF], f32, tag="mm", name=f"y{m}_{h}")
                   for m in range(KC)]
            for kc in range(KC):
                v = qkvT[6 + kc]

                def bcast(off, _h=h, _kc=kc):
                    # one pb PSUM tile live at a time (pb pool: 2 slots).
                    # Lane 0: DVE multiplies straight from PSUM (1x read).
                    # Lane 1: ACT evacuates to bf16 SBUF first so the DVE
                    # multiply runs in 2x mode - balances ACT vs DVE load.
                    pbps = pbpool.tile([P, NHALF], f32, tag="pb",
                                       name=f"pb{_kc}_{off}_{_h}")
                    nc.tensor.matmul(
                        pbps,
                        lhsT=ind6T[:, P * _kc:P * (_kc + 1)],
                        rhs=p_half[_h][off],
                        start=True, stop=True,
                    )
                    if _h == 0:
                        return pbps
                    pbs = avpool.tile([P, NHALF], bf16, tag="pbs")
                    nc.scalar.copy(pbs, pbps)
                    return pbs

                dL = dLs[kc]
                pb = bcast(0)
                m1 = avpool.tile([P, NHALF], bf16, tag="m")
                nc.vector.tensor_mul(m1, pb, dL[:, lo:hi])
                pb = bcast(2)
                m2 = avpool.tile([P, NHALF], bf16, tag="m")
                nc.vector.tensor_mul(m2, pb, dL[:, lo + 1:hi + 1])
                s12 = avpool.tile([P, NHALF], bf16, tag="m")
                nc.vector.tensor_sub(s12, m1, m2)
                nc.vector.tensor_add(half(attn[kc], h), s12, v[:, lo:hi])

                for m in range(KC):
                    nc.tensor.matmul(
                        yps[m],
                        lhsT=w_proj[kc][:, P * m:P * (m + 1)],
                        rhs=half(attn[kc], h),
                        start=(kc == 0), stop=False,
                    )

            for m in range(KC):
                nc.tensor.matmul(
                    yps[m],
                    lhsT=bias[:, P * m:P * (m + 1)],
                    rhs=half(ones, h),
                    start=False, stop=True,
                )
                yt = ypool.tile([P, NHALF], bf16, tag="y")
                nc.scalar.copy(yt, yps[m])
                nc.sync.dma_start(
                    out=d_yT[P * m:P * (m + 1), NHALF * h:NHALF * (h + 1)],
                    in_=yt)

    nc.compile()
    return nc


def _host_inputs(x, qkv_w, proj_w, proj_b):
    import ml_dtypes
    bf = ml_dtypes.bfloat16

    qkv_wT = np.ascontiguousarray(qkv_w.astype(np.float32).T).astype(bf)
    proj_wT = np.ascontiguousarray(proj_w.astype(np.float32).T).astype(bf)
    bias = proj_b.astype(np.float32).reshape(1, C).astype(bf)
    # head indicator: row p of chunk kc belongs to head 2*kc + p//64
    ind6 = np.zeros((P, 6 * KC), np.float32)
    ind6T = np.zeros((H, P * KC), np.float32)
    for kc in range(KC):
        for p in range(P):
            ind6[p, 6 * kc + 2 * kc + p // HD] = 1.0
            ind6T[2 * kc + p // HD, P * kc + p] = 1.0
    shared = {
        "qkv_wT": qkv_wT,
        "proj_wT": proj_wT,
        "proj_b": bias,
        "ind6": ind6.astype(bf),
        "ind6T": ind6T.astype(bf),
        "ones": np.ones((1, N), bf),
    }
    in_maps = []
    for b in range(B):
        m = dict(shared)
        m["xT"] = np.ascontiguousarray(x[b].astype(np.float32).T).astype(bf)
        in_maps.append(m)
    return in_maps


def kernel(x, qkv_w, proj_w, proj_b, _trace=False):
    from concourse import bass_utils

    x = np.asarray(x)
    if "nc" not in _cached:
        _cached["nc"] = _build_nc()
    nc = _cached["nc"]
    in_maps = _host_inputs(x, np.asarray(qkv_w), np.asarray(proj_w),
                           np.asarray(proj_b))
    res = bass_utils.run_bass_kernel_spmd(
        nc, in_maps, core_ids=list(range(NCORES)), trace=_trace)
    out = np.empty((B, N, C), np.float32)
    for b in range(B):
        out[b] = res.results[b]["yT"].astype(np.float32).T
    if _trace:
        _cached["last_result"] = res
    return out

